# Initial kernel scaffold
#
"""Bass/Tile TRN2 kernel for nn_Block_26001732010180 (TimeSformer-style block).

Sharding (8 cores): core i -> (b = i//2, khalf = i%2 = thalf).
Temporal stage: core handles batch b, seqs k in [98*khalf, 98*khalf+98), all T=16.
  Token order "(t, s)": col j = t*98 + s (t global, s local seq index).
Pair AllGather exchanges xt_full between the two cores of a batch; spatial
stage: core handles its 8 t's (t0 = 8*thalf), all 196 k (+ CLS).  The t-half /
b selection is done with per-core 0/1 multiplier input tensors so the emitted
program is identical on all 8 cores (SPMD requirement).

Activations chained feature-major ([C, tok], C on partitions) so GEMMs need no
activation transposes; LayerNorm runs token-major, followed by a PE-transpose.
Attention uses 8-seq block-diagonal batching (temporal, T<=16) with a static
mask, exp without max-subtraction (scores are small), and a ones-column in V
to get softmax denominators from the AV matmul.  The multi-scale window fusion
is folded into pre-scaled projection-weight variants and combined biases
(computed on host).
"""
import sys
sys.path.insert(0, '/opt/trn_rl_repo')
import numpy as np
from contextlib import ExitStack

import ml_dtypes
import concourse.bass as bass
import concourse.bacc as bacc
import concourse.tile as tile
from concourse import mybir
from concourse.bass_utils import run_bass_kernel_spmd

F32 = mybir.dt.float32
BF16 = mybir.dt.bfloat16
AF = mybir.ActivationFunctionType
OP = mybir.AluOpType
P = 128

B, T, K, C = 4, 16, 196, 768
NH, HD, MLP = 12, 64, 3072
SC = 98              # seqs per core (temporal)
NT = SC * T          # 1568 temporal tokens per core
NS = 8 * (K + 1)     # 1576 spatial tokens per core (8 t x 197)
NM = NS + 1          # 1577 mlp rows (last = true CLS row)
C8_0 = 8 * SC        # 784: first (t,s) col of the x8 range
C4_0 = 12 * SC       # 1176: first col of the x4 range
N_CORES = 8
EPS = 1e-5
SCALE = HD ** -0.5

_cache = {}

# temporal groups: 12 groups of 8 seqs + 1 tail group of 2 seqs; token col
# (group-major) = gbase + t*nseq + s.  scale-8 local col = g8 + (t-8)*nseq + s.
GROUPS = [(g, g * 128, 8) for g in range(12)] + [(12, 1536, 2)]
G8BASE = [g * 64 for g in range(12)] + [768]
G4BASE = [g * 32 for g in range(12)] + [384]


def row_tiles(n):
    return [(i, i * P, min(P, n - i * P)) for i in range((n + P - 1) // P)]


# ---------------------------------------------------------------------------
# emission helpers
# ---------------------------------------------------------------------------

def emit_ln(nc, pools, n_rows, load_tile_fn, out_fm, ident):
    """LayerNorm (no affine, eps=1e-5) over C=768; out feature-major bf16.

    load_tile_fn(ti, r0, nr) -> [128, 768] f32 sbuf view.
    out_fm: [128, 6, ncols] bf16 (c = kc*128 + p, col = token index).
    """
    stat, tmp, psT = pools['stat'], pools['tmp'], pools['psT']
    eps = pools['eps']
    for ti, r0, nr in row_tiles(n_rows):
        xt = load_tile_fn(ti, r0, nr)
        st = stat.tile([P, 3, 6], F32, tag="ln_st")
        xr = xt[:nr].rearrange("p (a b) -> p a b", a=3)
        for sg in range(3):
            nc.vector.bn_stats(out=st[:nr, sg, :], in_=xr[:, sg, :])
        mv = stat.tile([P, 2], F32, tag="ln_mv")
        nc.vector.bn_aggr(out=mv[:nr], in_=st[:nr])
        rstd = stat.tile([P, 1], F32, tag="ln_rstd")
        nc.scalar.activation(out=rstd[:nr], in_=mv[:nr, 1:2], func=AF.Sqrt,
                             bias=eps[:nr])
        nc.vector.reciprocal(out=rstd[:nr], in_=rstd[:nr])
        xn = tmp.tile([P, C], BF16, tag="ln_out")
        nc.vector.tensor_scalar(out=xn[:nr], in0=xt[:nr],
                                scalar1=mv[:nr, 0:1], scalar2=rstd[:nr],
                                op0=OP.subtract, op1=OP.mult)
        for kc in range(6):
            pt = psT.tile([P, P], BF16, tag="ln_tp")
            nc.tensor.transpose(pt[:, :nr], xn[:nr, kc * P:(kc + 1) * P],
                                ident[:nr, :nr])
            nc.any.tensor_copy(out=out_fm[:, kc, r0:r0 + nr], in_=pt[:, :nr])


def emit_gemm_fm(nc, pools, w_sb, act_fm, ncols, mtiles, evict_fn,
                 act_col_off=0, psum_tag="gfm"):
    """feature-major GEMM: psum[m, col] = sum_kc w[:,kc,m].T @ act[:,kc,col]."""
    ps = pools['ps']
    for mi in range(mtiles):
        for c0 in range(0, ncols, 512):
            ncl = min(512, ncols - c0)
            pp = ps.tile([P, 512], F32, tag=psum_tag)
            for kc in range(6):
                nc.tensor.matmul(
                    pp[:, :ncl],
                    w_sb[:, kc, mi * P:(mi + 1) * P],
                    act_fm[:, kc, act_col_off + c0:act_col_off + c0 + ncl],
                    start=(kc == 0), stop=(kc == 5))
            evict_fn(mi, c0, ncl, pp[:, :ncl])


def build_program(debug=False, n_reps=1, single=False):
    ncores = 1 if single else N_CORES
    nc = bacc.Bacc("TRN2", target_bir_lowering=False, debug=False,
                   enable_asserts=True, num_devices=ncores)
    env = {'debug': debug, 'nc': nc, 'single': single}

    def inp(name, shape, dt=BF16):
        h = nc.dram_tensor(name, shape, dt, kind="ExternalInput")
        env[name + '_d'] = h
        return h

    inp("xt", [NT, C], F32)
    inp("xtb", [NT, C], F32)
    inp("cls", [1, C], F32)
    inp("mth", [P, 2], F32)
    inp("mb", [P, B], F32)
    for s in ("16", "8", "4"):
        inp(f"wqk{s}", [C, 2 * C]); inp(f"wv{s}", [C, C])
    for nm in ("wp16a", "wp16b", "wp8b", "wp8c", "wp4c"):
        inp(nm, [C, C])
    inp("bias_abc", [3, C], F32)
    inp("wtfc", [C, C])
    inp("wqks", [C, 2 * C]); inp("wvs", [C, C])
    inp("wprojs", [C, C]); inp("bprojs_b", [P, C], F32)
    inp("wfc1", [C, MLP]); inp("bfc1", [1, MLP], F32)
    inp("wfc2", [MLP, C]); inp("bfc2b", [P, C], F32)

    identf_np = np.eye(P, dtype=np.float32)
    identb_np = np.eye(P, dtype=ml_dtypes.bfloat16)
    mask8_np = np.where((np.arange(P)[:, None] % 8) == (np.arange(P)[None, :] % 8),
                        1.0, 0.0).astype(ml_dtypes.bfloat16)
    mask2_np = np.where((np.arange(32)[:, None] % 2) == (np.arange(32)[None, :] % 2),
                        1.0, 0.0).astype(ml_dtypes.bfloat16)
    env['identf_c'] = nc.inline_tensor(identf_np, name="identfc")
    env['identb_c'] = nc.inline_tensor(identb_np, name="identbc")
    env['mask8_c'] = nc.inline_tensor(mask8_np, name="mask8c")
    env['mask2_c'] = nc.inline_tensor(mask2_np, name="mask2c")

    env['out_d'] = nc.dram_tensor("out", [NM, C], F32, kind="ExternalOutput")
    dbg = {}
    if debug:
        for nm, shp in [("d_xtfull", [NT, C]), ("d_xspre", [NS, C]),
                        ("d_ressp", [NS, C]), ("d_clsagg", [1, C]),
                        ("d_xcat", [NM, C])]:
            dbg[nm] = nc.dram_tensor(nm, shp, F32, kind="ExternalOutput")
    env['dbg'] = dbg

    with tile.TileContext(nc) as tc, ExitStack() as ctx:
        env['tc'] = tc
        consts = ctx.enter_context(tc.tile_pool(name="consts", bufs=1))
        tmp = ctx.enter_context(tc.tile_pool(name="tmp", bufs=4))
        stat = ctx.enter_context(tc.tile_pool(name="stat", bufs=4))
        small = ctx.enter_context(tc.tile_pool(name="small", bufs=1))
        psM = ctx.enter_context(tc.tile_pool(name="psM", bufs=2, space="PSUM"))
        psS = ctx.enter_context(tc.tile_pool(name="psS", bufs=2, space="PSUM"))
        psO = ctx.enter_context(tc.tile_pool(name="psO", bufs=2, space="PSUM"))
        psT = ctx.enter_context(tc.tile_pool(name="psT", bufs=2, space="PSUM"))
        dram = ctx.enter_context(tc.tile_pool(name="dram", bufs=1, space="DRAM"))
        env['pools'] = {'ps': psM, 'psT': psT, 'psA': psS, 'psO': psO,
                        'stat': stat, 'tmp': tmp, 'small': small}

        for nm, src, shp, dt in [
                ('identf', 'identf_c', [P, P], F32),
                ('identb', 'identb_c', [P, P], BF16),
                ('mask8', 'mask8_c', [P, P], BF16),
                ('mask2', 'mask2_c', [32, 32], BF16)]:
            t = consts.tile(shp, dt, tag=nm, name=nm)
            nc.sync.dma_start(t, env[src].ap())
            env[nm] = t
        for nm, src, shp in [('mth', 'mth_d', [P, 2]), ('mb', 'mb_d', [P, B]),
                             ('cls_sb', 'cls_d', [1, C]),
                             ('bprojs_b', 'bprojs_b_d', [P, C]),
                             ('bfc2b', 'bfc2b_d', [P, C])]:
            t = consts.tile(shp, F32, tag=nm, name=nm)
            nc.sync.dma_start(t, env[src].ap())
            env[nm] = t
        epst = consts.tile([P, 1], F32, tag="eps", name="epst")
        nc.vector.memset(epst, EPS)
        env['eps'] = epst
        env['pools']['eps'] = epst
        t = consts.tile([P, 3, 6], F32, tag="bias_abc")
        nc.sync.dma_start(t, env['bias_abc_d'].ap().rearrange(
            "a (kc p) -> p a kc", p=P))
        env['bias_abc'] = t
        t = consts.tile([P, 24], F32, tag="bfc1")
        nc.sync.dma_start(t, env['bfc1_d'].ap().rearrange(
            "o (kc p) -> p (o kc)", p=P))
        env['bfc1'] = t

        def load_w(pool, name, cols, tag, bufs=1):
            w = pool.tile([P, 6, cols], BF16, tag=tag, name="w_" + name,
                          bufs=bufs)
            nc.sync.dma_start(w, env[name + '_d'].ap().rearrange(
                "(kc p) n -> p kc n", p=P))
            return w
        env['load_w'] = load_w
        env['xcat_d'] = dram.tile([NM, C], F32, name='xcat_d')

        env['bounce'] = dram.tile([NT, C], F32, name='bounce')
        env['gath'] = dram.tile([2, NT, C], F32, name='gath')
        env['cls_bounce'] = dram.tile([8, C], F32, name='cls_bounce')
        env['cls_gath'] = nc.dram_tensor("clsg", [8 * N_CORES, C], F32,
                                         addr_space="Shared")
        env['xspre_d'] = dram.tile([NS, C], F32, name='xspre')

        for _rep in range(n_reps):
            emit_body(nc, env)
        _cache['marks'] = list(env.get('marks', []))

    nc.compile()
    return nc, dbg


def emit_body(nc, env):
    g = env.__getitem__
    tc = g('tc')
    pools = g('pools')
    tmp, stat, small = pools['tmp'], pools['stat'], pools['small']
    psM, psS, psO, psT = pools['ps'], pools['psA'], pools['psO'], pools['psT']
    identf, identb, mask8, mask2 = g('identf'), g('identb'), g('mask8'), g('mask2')
    mth, mb, cls_sb = g('mth'), g('mb'), g('cls_sb')
    bias_abc, bprojs_b, bfc1, bfc2b = (g('bias_abc'), g('bprojs_b'), g('bfc1'),
                                       g('bfc2b'))
    load_w = g('load_w')
    bounce, gath, cls_bounce, cls_gath, xspre_d, xcat_d = (
        g('bounce'), g('gath'), g('cls_bounce'), g('cls_gath'), g('xspre_d'),
        g('xcat_d'))
    out_d, dbg, debug = g('out_d'), g('dbg'), g('debug')

    marks = env.setdefault('marks', [])

    def mark(lbl):
        marks.append((lbl, nc.next_id()))
    mark('t_ln')

    # ================= temporal stage =================
    with tc.tile_pool(name="TPo", bufs=1) as TPo:
        o = {}
        for nm, cols in (("16", NT), ("8", 784), ("4", 392)):
            o[nm] = TPo.tile([P, 6, cols], BF16, tag=f"o{nm}", name=f"o{nm}")

        with tc.tile_pool(name="TPln", bufs=1) as TPln:
            xtn = TPln.tile([P, 6, NT], BF16, tag="xtn")

            def load_xt(ti, r0, nr):
                t = tmp.tile([P, C], F32, tag="t768")
                nc.sync.dma_start(t[:nr], g('xt_d').ap()[r0:r0 + nr])
                return t
            emit_ln(nc, pools, NT, load_xt, xtn, identb)

            # block-rotated qk buffers: all scales in one scope; attention of
            # block b overlaps the qk GEMM of block b+1.
            sc_tab = (("16", 0, 16, lambda gi: GROUPS[gi][1]),
                      ("8", 8, 8, lambda gi: G8BASE[gi]),
                      ("4", 12, 4, lambda gi: G4BASE[gi]))
            gblocks = [list(range(4 * i, 4 * i + 4)) for i in range(3)] + [[12]]
            with tc.tile_pool(name="TPqk", bufs=1) as TPqk:
                for nm, tlo, nt_sc, lbase in sc_tab:
                    mark('t_scale' + nm)
                    w = load_w(TPqk, f'wqk{nm}', 2 * C, tag="wqk_t", bufs=2)
                    wv = load_w(TPqk, f'wv{nm}', C, tag="wv_t", bufs=2)
                    for gs in gblocks:
                        bw = sum(nt_sc * GROUPS[gi][2] for gi in gs)
                        c0 = GROUPS[gs[0]][1] + tlo * GROUPS[gs[0]][2]
                        # xtn source cols for this scale+block are contiguous
                        # only per group; emit per-group matmuls into one psum
                        qkb = TPqk.tile([P, 12, 512], BF16, tag="qkb",
                                        bufs=3, name="qkb")
                        for mi in range(12):
                            pp = psM.tile([P, 512], F32, tag="mm")
                            off = 0
                            offs = []
                            for gi in gs:
                                _, gb, nseq = GROUPS[gi]
                                wcols = nt_sc * nseq
                                for kc in range(6):
                                    nc.tensor.matmul(
                                        pp[:, off:off + wcols],
                                        w[:, kc, mi * P:(mi + 1) * P],
                                        xtn[:, kc, gb + tlo * nseq:
                                            gb + tlo * nseq + wcols],
                                        start=(kc == 0), stop=(kc == 5))
                                offs.append((off, wcols, gi))
                                off += wcols
                            nc.vector.tensor_copy(out=qkb[:, mi, :bw],
                                                  in_=pp[:, :bw])
                        boff = {gi: ofs for (ofs, wcols, gi) in offs}
                        for gi in gs:
                            _, gb, nseq = GROUPS[gi]
                            kp = nt_sc * nseq
                            src0 = gb + tlo * nseq
                            va = tmp.tile([P, 12, HD + 1], BF16, tag="vaug")
                            for nb in range(2):
                                pv = psM.tile([P, 512], F32, tag="mm")
                                for kc in range(6):
                                    nc.tensor.matmul(
                                        pv[:kp, :384],
                                        xtn[:, kc, src0:src0 + kp],
                                        wv[:, kc, nb * 384:(nb + 1) * 384],
                                        start=(kc == 0), stop=(kc == 5))
                                nc.vector.tensor_copy(
                                    out=va[:kp, 6 * nb:6 * nb + 6, 0:HD],
                                    in_=pv[:kp, :384].rearrange(
                                        "p (h d) -> p h d", d=HD))
                            nc.vector.memset(va[:kp, :, HD:HD + 1], 1.0)
                            lq0 = boff[gi]
                            lc0 = lbase(gi)
                            msk = (mask8 if nseq == 8 else mask2)[:kp, :kp]
                            for h in range(NH):
                                mt_q, ro = h // 2, (h % 2) * HD
                                pS = psS.tile([P, 256], F32, tag="sps")
                                nc.tensor.matmul(
                                    pS[:kp, :kp],
                                    qkb[ro:ro + HD, 6 + mt_q, lq0:lq0 + kp],
                                    qkb[ro:ro + HD, mt_q, lq0:lq0 + kp],
                                    start=True, stop=True)
                                pt = tmp.tile([P, 256], BF16, tag="ptile")
                                nc.scalar.activation(out=pt[:kp, :kp],
                                                     in_=pS[:kp, :kp],
                                                     func=AF.Exp, scale=SCALE)
                                nc.vector.tensor_mul(out=pt[:kp, :kp],
                                                     in0=pt[:kp, :kp], in1=msk)
                                pO = psO.tile([HD + 1, 256], F32, tag="ops")
                                nc.tensor.matmul(pO[:, :kp], va[:kp, h, :],
                                                 pt[:kp, :kp], start=True,
                                                 stop=True)
                                rec = small.tile([1, 256], F32, tag="rec",
                                                 bufs=3)
                                nc.vector.reciprocal(out=rec[:, :kp],
                                                     in_=pO[HD:HD + 1, :kp])
                                rb = small.tile([HD, 256], F32, tag="rb",
                                                bufs=3)
                                nc.gpsimd.partition_broadcast(rb[:, :kp],
                                                              rec[:, :kp])
                                nc.vector.tensor_mul(
                                    out=o[nm][ro:ro + HD, mt_q, lc0:lc0 + kp],
                                    in0=pO[0:HD, :kp], in1=rb[:, :kp])

        with tc.tile_pool(name="TPpr", bufs=1) as TPpr:
            mark('t_proj')
            wp = {}
            for nm in ("wp16a", "wp16b", "wp8b", "wp8c", "wp4c"):
                wp[nm] = load_w(TPpr, nm, C, tag=f"wp_{nm}")
            res = TPpr.tile([P, 6, NT], BF16, tag="res")
            # per (mtile, 4-group block): accumulate the three scales in psum,
            # then evict with the 3 range biases per group.
            for mi in range(6):
                for g0 in range(0, 13, 4):
                    gs = list(range(g0, min(g0 + 4, 13)))
                    blk0 = GROUPS[g0][1]
                    blkw = sum(16 * GROUPS[gi][2] for gi in gs)
                    pp = psM.tile([P, 512], F32, tag="mm")
                    mms = []
                    for gi in gs:
                        _, gb, nseq = GROUPS[gi]
                        ofs = gb - blk0
                        # x16 full group (bias range a for t<8, b for t>=8)
                        mms.append((wp['wp16a'], o['16'], gb, ofs, 8 * nseq))
                        mms.append((wp['wp16b'], o['16'], gb + 8 * nseq,
                                    ofs + 8 * nseq, 8 * nseq))
                        mms.append((wp['wp8b'], o['8'], G8BASE[gi],
                                    ofs + 8 * nseq, 4 * nseq))
                        mms.append((wp['wp8c'], o['8'], G8BASE[gi] + 4 * nseq,
                                    ofs + 12 * nseq, 4 * nseq))
                        mms.append((wp['wp4c'], o['4'], G4BASE[gi],
                                    ofs + 12 * nseq, 4 * nseq))
                    n_mm = 6 * len(mms)
                    i_mm = 0
                    for (w, osrc, sc0, dof, wcols) in mms:
                        first = w is wp['wp16a'] or w is wp['wp16b']
                        for kc in range(6):
                            nc.tensor.matmul(
                                pp[:, dof:dof + wcols],
                                w[:, kc, mi * P:(mi + 1) * P],
                                osrc[:, kc, sc0:sc0 + wcols],
                                start=(first and kc == 0),
                                stop=(i_mm == n_mm - 1),
                                skip_group_check=True)
                            i_mm += 1
                    full = [gi for gi in gs if GROUPS[gi][2] == 8]
                    for rng, t0, t1 in ((0, 0, 8), (1, 8, 12), (2, 12, 16)):
                        if full:
                            nf = len(full)
                            rv = res[:, mi, 0:1536].rearrange(
                                "p (gg x) -> p gg x", x=P)
                            pv = pp[:, 0:512].rearrange(
                                "p (gg x) -> p gg x", x=P)
                            nc.scalar.activation(
                                out=rv[:, full[0]:full[0] + nf,
                                       t0 * 8:t1 * 8],
                                in_=pv[:, (GROUPS[full[0]][1] - blk0) // P:
                                       (GROUPS[full[0]][1] - blk0) // P + nf,
                                       t0 * 8:t1 * 8],
                                func=AF.Identity,
                                bias=bias_abc[:, rng, mi:mi + 1])
                        for gi in gs:
                            _, gb, nseq = GROUPS[gi]
                            if nseq == 8:
                                continue
                            ofs = gb - blk0
                            nc.scalar.activation(
                                out=res[:, mi, gb + t0 * nseq:gb + t1 * nseq],
                                in_=pp[:, ofs + t0 * nseq:ofs + t1 * nseq],
                                func=AF.Identity,
                                bias=bias_abc[:, rng, mi:mi + 1])

            mark('t_tfc')
            wtfc = load_w(TPpr, 'wtfc', C, tag="wtfc")
            for ti, r0, nr in row_tiles(NT):
                xb = tmp.tile([P, C], F32, tag="t768")
                nc.sync.dma_start(xb[:nr], g('xtb_d').ap()[r0:r0 + nr])
                xf = tmp.tile([P, C], F32, tag="t768")
                for nb in range(2):
                    pt = psM.tile([P, 512], F32, tag="mm")
                    for kc in range(6):
                        nc.tensor.matmul(pt[:nr, :384],
                                         res[:, kc, r0:r0 + nr],
                                         wtfc[:, kc, nb * 384:(nb + 1) * 384],
                                         start=(kc == 0), stop=(kc == 5))
                    nc.vector.tensor_add(
                        out=xf[:nr, nb * 384:(nb + 1) * 384], in0=pt[:nr, :384],
                        in1=xb[:nr, nb * 384:(nb + 1) * 384])
                nc.sync.dma_start(bounce[r0:r0 + nr], xf[:nr])
                if debug:
                    nc.sync.dma_start(dbg['d_xtfull'].ap()[r0:r0 + nr],
                                      xf[:nr])

    mark('exchange')
    if g('single'):
        for rk in range(2):
            nc.sync.dma_start(gath[rk], bounce[:])
    else:
        nc.gpsimd.collective_compute(
            "AllGather", OP.bypass,
            replica_groups=[[0, 1], [2, 3], [4, 5], [6, 7]],
            ins=[bounce.opt()], outs=[gath.opt()])

    # ============ xs_pre (DRAM): rows (tl, 0)=cls, (tl, 1+k)=xt_full ============
    mark('xspre')
    _XS_cm = tc.tile_pool(name="XS", bufs=4)
    XS = _XS_cm.__enter__()
    for tl in range(8):
        nc.sync.dma_start(xspre_d[tl * 197:tl * 197 + 1], cls_sb)
        for rank in range(2):
            # candidate rows for t = tl (lo) and t = 8+tl (hi); bounce rows
            # are group-major: row = g*128 + t*nseq + s  (k = 8g+s ascending)
            lo = XS.tile([SC, C], F32, tag="sel_lo", bufs=4)
            hi = XS.tile([SC, C], F32, tag="sel_hi", bufs=4)
            gmain = gath[rank][0:1536].rearrange("(gg x) c -> gg x c", x=P)
            for dst, t in ((lo, tl), (hi, 8 + tl)):
                nc.sync.dma_start(dst[:96], gmain[:, t * 8:t * 8 + 8, :])
                nc.sync.dma_start(dst[96:98],
                                  gath[rank][1536 + t * 2:1536 + t * 2 + 2])
            selt = XS.tile([SC, C], F32, tag="sel_out", bufs=4)
            nc.vector.tensor_scalar_mul(out=selt, in0=hi,
                                        scalar1=mth[:SC, 1:2])
            nc.vector.scalar_tensor_tensor(out=selt, in0=lo,
                                           scalar=mth[:SC, 0:1], in1=selt,
                                           op0=OP.mult, op1=OP.add)
            nc.sync.dma_start(
                xspre_d[tl * 197 + 1 + rank * SC:tl * 197 + 1 + rank * SC + SC],
                selt)
    _XS_cm.__exit__(None, None, None)
    if debug:
        for ti, r0, nr in row_tiles(NS):
            t = tmp.tile([P, C], F32, tag="t768")
            nc.sync.dma_start(t[:nr], xspre_d[r0:r0 + nr])
            nc.sync.dma_start(dbg['d_xspre'].ap()[r0:r0 + nr], t[:nr])

    # ================= spatial stage =================
    mark('s_ln')
    with tc.tile_pool(name="SP1", bufs=1) as SP1:
        ressp = SP1.tile([P, 13, C], F32, tag="ressp")
        with tc.tile_pool(name="SPo", bufs=1) as SPo:
            os_ = SPo.tile([P, 6, NS], BF16, tag="osp", name="os_")
            with tc.tile_pool(name="SPln", bufs=1) as SPln:
                xsn = SPln.tile([P, 6, NS], BF16, tag="xsn")

                def load_xs(ti, r0, nr):
                    t = tmp.tile([P, C], F32, tag="t768")
                    nc.sync.dma_start(t[:nr], xspre_d[r0:r0 + nr])
                    return t
                emit_ln(nc, pools, NS, load_xs, xsn, identb)

                mark('s_qkv')
                with tc.tile_pool(name="SPqk", bufs=1) as SPqk:
                    w = load_w(SPqk, 'wqks', 2 * C, tag="wqk_t", bufs=1)
                    wvs = load_w(SPqk, 'wvs', C, tag="wv_t", bufs=1)
                    for tl in range(8):
                        q0 = tl * 197
                        qkb = SPqk.tile([P, 12, 197], BF16, tag="qksb",
                                        bufs=3, name="qkb_sp")
                        for mi in range(12):
                            pp = psM.tile([P, 512], F32, tag="mm")
                            for kc in range(6):
                                nc.tensor.matmul(
                                    pp[:, :197],
                                    w[:, kc, mi * P:(mi + 1) * P],
                                    xsn[:, kc, q0:q0 + 197],
                                    start=(kc == 0), stop=(kc == 5))
                            nc.vector.tensor_copy(out=qkb[:, mi, :],
                                                  in_=pp[:, :197])
                        va = tmp.tile([P, 2, 12, HD + 1], BF16, tag="vaug")
                        for kt in range(2):
                            kp = 128 if kt == 0 else 69
                            for nb in range(2):
                                pv = psM.tile([P, 512], F32, tag="mm")
                                for kc in range(6):
                                    nc.tensor.matmul(
                                        pv[:kp, :384],
                                        xsn[:, kc, q0 + kt * P:
                                            q0 + kt * P + kp],
                                        wvs[:, kc, nb * 384:(nb + 1) * 384],
                                        start=(kc == 0), stop=(kc == 5))
                                nc.vector.tensor_copy(
                                    out=va[:kp, kt, 6 * nb:6 * nb + 6, 0:HD],
                                    in_=pv[:kp, :384].rearrange(
                                        "p (h d) -> p h d", d=HD))
                            nc.vector.memset(va[:kp, kt, :, HD:HD + 1], 1.0)
                        for h in range(NH):
                            mt_q, ro = h // 2, (h % 2) * HD
                            pO = psO.tile([HD + 1, 256], F32, tag="ops")
                            for kt in range(2):
                                kp = 128 if kt == 0 else 69
                                pS = psS.tile([P, 256], F32, tag="sps")
                                nc.tensor.matmul(
                                    pS[:kp, :197],
                                    qkb[ro:ro + HD, 6 + mt_q,
                                        kt * P:kt * P + kp],
                                    qkb[ro:ro + HD, mt_q, :],
                                    start=True, stop=True)
                                pt = tmp.tile([P, 256], BF16, tag="ptile")
                                nc.scalar.activation(out=pt[:kp, :197],
                                                     in_=pS[:kp, :197],
                                                     func=AF.Exp, scale=SCALE)
                                nc.tensor.matmul(pO[:, :197],
                                                 va[:kp, kt, h, :],
                                                 pt[:kp, :197],
                                                 start=(kt == 0),
                                                 stop=(kt == 1))
                            rec = small.tile([1, 256], F32, tag="rec", bufs=3)
                            nc.vector.reciprocal(out=rec[:, :197],
                                                 in_=pO[HD:HD + 1, :197])
                            rb = small.tile([HD, 256], F32, tag="rb", bufs=3)
                            nc.gpsimd.partition_broadcast(rb[:, :197],
                                                          rec[:, :197])
                            nc.vector.tensor_mul(
                                out=os_[ro:ro + HD, mt_q, q0:q0 + 197],
                                in0=pO[0:HD, :197], in1=rb[:, :197])

            mark('s_proj')
            with tc.tile_pool(name="SPp", bufs=1) as SPp:
                wprojs = load_w(SPp, 'wprojs', C, tag="wproj")
                for ti, r0, nr in row_tiles(NS):
                    for nb in range(2):
                        pp = psM.tile([P, 512], F32, tag="mm")
                        for kc in range(6):
                            nc.tensor.matmul(
                                pp[:nr, :384], os_[:, kc, r0:r0 + nr],
                                wprojs[:, kc, nb * 384:(nb + 1) * 384],
                                start=(kc == 0), stop=(kc == 5))
                        nc.vector.tensor_add(
                            out=ressp[:nr, ti, nb * 384:(nb + 1) * 384],
                            in0=pp[:nr, :384],
                            in1=bprojs_b[:nr, nb * 384:(nb + 1) * 384])
                if debug:
                    for ti, r0, nr in row_tiles(NS):
                        nc.sync.dma_start(dbg['d_ressp'].ap()[r0:r0 + nr],
                                          ressp[:nr, ti, :])

        # ================= CLS aggregation =================
        mark('cls')
        for tl in range(8):
            r = tl * 197
            nc.sync.dma_start(cls_bounce[tl:tl + 1],
                              ressp[r % P:r % P + 1, r // P, :])
        if g('single'):
            for rk in range(8):
                nc.sync.dma_start(cls_gath.ap()[8 * rk:8 * rk + 8],
                                  cls_bounce[:])
        else:
            nc.gpsimd.collective_compute(
                "AllGather", OP.bypass,
                replica_groups=[list(range(N_CORES))],
                ins=[cls_bounce.opt()], outs=[cls_gath.ap()])
        cls_own = small.tile([16, C], F32, tag="clsown")
        for j in range(B):
            cj = tmp.tile([16, C], F32, tag="t768")
            nc.sync.dma_start(cj, cls_gath.ap()[16 * j:16 * j + 16])
            if j == 0:
                nc.vector.tensor_scalar_mul(out=cls_own, in0=cj,
                                            scalar1=mb[:16, 0:1])
            else:
                nc.vector.scalar_tensor_tensor(out=cls_own, in0=cj,
                                               scalar=mb[:16, j:j + 1],
                                               in1=cls_own, op0=OP.mult,
                                               op1=OP.add)
        st = stat.tile([16, 3, 6], F32, tag="ln_st")
        cr = cls_own.rearrange("p (a b) -> p a b", a=3)
        for sg in range(3):
            nc.vector.bn_stats(out=st[:, sg, :], in_=cr[:, sg, :])
        mv = stat.tile([16, 2], F32, tag="ln_mv")
        nc.vector.bn_aggr(out=mv, in_=st)
        rstd = stat.tile([16, 1], F32, tag="ln_rstd")
        nc.scalar.activation(out=rstd, in_=mv[:, 1:2], func=AF.Sqrt,
                             bias=g('eps')[:16])
        nc.vector.reciprocal(out=rstd, in_=rstd)
        clsn = small.tile([16, C], F32, tag="clsn")
        nc.vector.tensor_scalar(out=clsn, in0=cls_own, scalar1=mv[:, 0:1],
                                scalar2=rstd, op0=OP.subtract, op1=OP.mult)
        clsfm = small.tile([P, 6, 16], F32, tag="clsfm")
        for kc in range(6):
            ptc = psT.tile([P, 16], F32, tag="ln_tp")
            nc.tensor.transpose(ptc, clsn[:, kc * P:(kc + 1) * P],
                                identf[:16, :16])
            nc.vector.tensor_copy(out=clsfm[:, kc, :], in_=ptc)
        psc = psS.tile([1, 256], F32, tag="sps")
        for kc in range(6):
            nc.tensor.matmul(psc[:, :16], clsfm[:, kc, 15:16], clsfm[:, kc, :],
                             start=(kc == 0), stop=(kc == 5))
        mx = small.tile([1, 1], F32, tag="clsmx")
        nc.vector.reduce_max(out=mx, in_=psc[:, :16], axis=mybir.AxisListType.X)
        sc_sb = small.tile([1, 16], F32, tag="clssb")
        nc.vector.tensor_scalar_sub(out=sc_sb, in0=psc[:, :16], scalar1=mx)
        esum = small.tile([1, 1], F32, tag="clses")
        att = small.tile([1, 16], F32, tag="clsatt")
        nc.scalar.activation(out=att, in_=sc_sb, func=AF.Exp, accum_out=esum)
        nc.vector.reciprocal(out=esum, in_=esum)
        nc.vector.tensor_scalar_mul(out=att, in0=att, scalar1=esum)
        attT_ps = psT.tile([16, 16], F32, tag="ln_tp")
        nc.tensor.transpose(attT_ps[:, :1], att, identf[:1, :1])
        attT = small.tile([16, 1], F32, tag="attTs")
        nc.vector.tensor_copy(out=attT, in_=attT_ps[:, :1])
        clsrow = small.tile([1, C], F32, tag="clsrow")
        for nb in range(2):
            pagg = psM.tile([P, 512], F32, tag="mm")
            nc.tensor.matmul(pagg[:1, :384], attT,
                             clsn[:, nb * 384:(nb + 1) * 384],
                             start=True, stop=True)
            nc.vector.tensor_add(out=clsrow[:, nb * 384:(nb + 1) * 384],
                                 in0=pagg[:1, :384],
                                 in1=cls_sb[:, nb * 384:(nb + 1) * 384])
        if debug:
            nc.sync.dma_start(dbg['d_clsagg'].ap(), clsrow)

        mark('xcat')
        # ========== x_cat -> DRAM ==========
        for ti, r0, nr in row_tiles(NS):
            t = tmp.tile([P, C], F32, tag="t768")
            nc.sync.dma_start(t[:nr], xspre_d[r0:r0 + nr])
            xc = tmp.tile([P, C], F32, tag="t768")
            nc.vector.tensor_add(out=xc[:nr], in0=t[:nr], in1=ressp[:nr, ti, :])
            nc.sync.dma_start(xcat_d[r0:r0 + nr], xc[:nr])
        nc.sync.dma_start(xcat_d[NM - 1:NM], clsrow)
    if debug:
        for ti, r0, nr in row_tiles(NM):
            t = tmp.tile([P, C], F32, tag="t768")
            nc.sync.dma_start(t[:nr], xcat_d[r0:r0 + nr])
            nc.sync.dma_start(dbg['d_xcat'].ap()[r0:r0 + nr], t[:nr])

    # ================= MLP =================
    mark('m_ln')
    with tc.tile_pool(name="ML", bufs=1) as ML:
        wfc1 = load_w(ML, 'wfc1', MLP, tag="wfc1")
        wfc2 = ML.tile([P, 24, C], BF16, tag="wfc2", name="wfc2")
        nc.sync.dma_start(wfc2, g('wfc2_d').ap().rearrange(
            "(kc p) n -> p kc n", p=P))
        halves = [(0, 544), (544, 544), (1088, NM - 1088)]
        for hi0, hw in halves:
            hn = ML.tile([P, 6, 544], BF16, tag="hn", bufs=2, name="hn")

            def load_xc(ti, r0, nr, hi0=hi0):
                t = tmp.tile([P, C], F32, tag="t768")
                nc.sync.dma_start(t[:nr], xcat_d[hi0 + r0:hi0 + r0 + nr])
                return t
            emit_ln(nc, pools, hw, load_xc, hn, identb)

            gfm = ML.tile([P, 24, 544], BF16, tag="gfm", bufs=2, name="gfm")
            for mi in range(24):
                for c0 in range(0, hw, 512):
                    ncl = min(512, hw - c0)
                    pg = psM.tile([P, 512], F32, tag="mm")
                    for kc in range(6):
                        nc.tensor.matmul(pg[:, :ncl],
                                         wfc1[:, kc, mi * P:(mi + 1) * P],
                                         hn[:, kc, c0:c0 + ncl],
                                         start=(kc == 0), stop=(kc == 5))
                    nc.scalar.activation(out=gfm[:, mi, c0:c0 + ncl],
                                         in_=pg[:, :ncl], func=AF.Gelu,
                                         bias=bfc1[:, mi:mi + 1])
            if hi0 == halves[-1][0]:
                mark('m_fc2')
            for ti, r0, nr in row_tiles(hw):
                xc = tmp.tile([P, C], F32, tag="t768")
                nc.sync.dma_start(xc[:nr], xcat_d[hi0 + r0:hi0 + r0 + nr])
                outt = tmp.tile([P, C], F32, tag="t768")
                for nb in range(2):
                    po = psM.tile([P, 512], F32, tag="mm")
                    for kc in range(24):
                        nc.tensor.matmul(po[:nr, :384],
                                         gfm[:, kc, r0:r0 + nr],
                                         wfc2[:, kc, nb * 384:(nb + 1) * 384],
                                         start=(kc == 0), stop=(kc == 23))
                    nc.vector.tensor_add(out=po[:nr, :384], in0=po[:nr, :384],
                                         in1=bfc2b[:nr, nb * 384:(nb + 1) * 384])
                    nc.vector.tensor_add(out=outt[:nr, nb * 384:(nb + 1) * 384],
                                         in0=po[:nr, :384],
                                         in1=xc[:nr, nb * 384:(nb + 1) * 384])
                nc.sync.dma_start(out_d.ap()[hi0 + r0:hi0 + r0 + nr],
                                  outt[:nr])


# ---------------------------------------------------------------------------
# host side
# ---------------------------------------------------------------------------

def prep_inputs(inputs):
    x = np.asarray(inputs['x'], np.float32)
    f32 = lambda a: np.ascontiguousarray(np.asarray(a, np.float32))
    tobf = lambda a: np.ascontiguousarray(
        np.asarray(a, np.float32).astype(ml_dtypes.bfloat16))

    Wqkv = {s: np.asarray(inputs[f'Wqkv{s}'], np.float32) for s in (4, 8, 16)}
    Wp = {s: np.asarray(inputs[f'Wp{s}'], np.float32) for s in (4, 8, 16)}
    bp = {s: np.asarray(inputs[f'bp{s}'], np.float32) for s in (4, 8, 16)}
    Wqkv_s = np.asarray(inputs['Wqkv_s'], np.float32)
    shared = {
        'wqk16': tobf(Wqkv[16][:, :2 * C]), 'wv16': tobf(Wqkv[16][:, 2 * C:]),
        'wqk8': tobf(Wqkv[8][:, :2 * C]), 'wv8': tobf(Wqkv[8][:, 2 * C:]),
        'wqk4': tobf(Wqkv[4][:, :2 * C]), 'wv4': tobf(Wqkv[4][:, 2 * C:]),
        'wp16a': tobf(Wp[16]), 'wp16b': tobf(0.5 * Wp[16]),
        'wp8b': tobf(0.5 * Wp[8]), 'wp8c': tobf(0.25 * Wp[8]),
        'wp4c': tobf(0.25 * Wp[4]),
        'bias_abc': f32(np.stack([bp[16], 0.5 * bp[16] + 0.5 * bp[8],
                                  0.5 * bp[16] + 0.25 * bp[8] + 0.25 * bp[4]])),
        'wtfc': tobf(inputs['Wtfc']),
        'wqks': tobf(Wqkv_s[:, :2 * C]), 'wvs': tobf(Wqkv_s[:, 2 * C:]),
        'wprojs': tobf(inputs['Wproj_s']),
        'bprojs_b': f32(np.tile(np.asarray(inputs['bproj_s'])[None, :], (P, 1))),
        'wfc1': tobf(inputs['Wfc1']),
        'bfc1': f32(np.asarray(inputs['bfc1']).reshape(1, MLP)),
        'wfc2': tobf(inputs['Wfc2']),
        'bfc2b': f32(np.tile(np.asarray(inputs['bfc2'])[None, :], (P, 1))),
    }
    btfc = np.asarray(inputs['btfc'], np.float32)
    perm = np.empty(NT, np.int64)
    for gi, gb, nseq in GROUPS:
        for t in range(T):
            for s in range(nseq):
                perm[gb + t * nseq + s] = (8 * gi + s) * T + t
    in_maps = []
    for core in range(N_CORES):
        b, half = core // 2, core % 2
        xs = x[b, 1 + half * NT: 1 + half * NT + NT]
        xt = xs[perm]
        mth = np.zeros((P, 2), np.float32); mth[:, half] = 1.0
        mbv = np.zeros((P, B), np.float32); mbv[:, b] = 1.0
        m = dict(shared)
        m.update({'xt': f32(xt), 'xtb': f32(xt + btfc),
                  'cls': f32(x[b, 0:1]), 'mth': mth, 'mb': mbv})
        in_maps.append(m)
    return in_maps


def assemble(results):
    out = np.zeros((B, 1 + K * T, C), np.float32)
    for core in range(N_CORES):
        b, half = core // 2, core % 2
        o = np.asarray(results[core]['out'])
        main = o[:NS].reshape(8, K + 1, C)[:, 1:, :]
        t = 8 * half + np.arange(8)
        idx = 1 + np.arange(K)[None, :] * T + t[:, None]
        out[b, idx.reshape(-1)] = main.reshape(-1, C)
        if half == 0:
            out[b, 0] = o[NM - 1]
    return out


def kernel(**inputs):
    if 'prog' not in _cache:
        _cache['prog'] = build_program(debug=False)
    nc, _ = _cache['prog']
    in_maps = prep_inputs(inputs)
    res = run_bass_kernel_spmd(nc, in_maps, list(range(N_CORES)))
    return assemble(res.results)



# revision 47
# speedup vs baseline: 1.1082x; 1.1082x over previous
"""Bass/Tile TRN2 kernel for nn_Block_26001732010180 (TimeSformer-style block).

Sharding (8 cores): core i -> (b = i//2, khalf = i%2 = thalf).
Temporal stage: core handles batch b, seqs k in [98*khalf, 98*khalf+98), all T=16.
  Token order "(t, s)": col j = t*98 + s (t global, s local seq index).
Pair AllGather exchanges xt_full between the two cores of a batch; spatial
stage: core handles its 8 t's (t0 = 8*thalf), all 196 k (+ CLS).  The t-half /
b selection is done with per-core 0/1 multiplier input tensors so the emitted
program is identical on all 8 cores (SPMD requirement).

Activations chained feature-major ([C, tok], C on partitions) so GEMMs need no
activation transposes; LayerNorm runs token-major, followed by a PE-transpose.
Attention uses 8-seq block-diagonal batching (temporal, T<=16) with a static
mask, exp without max-subtraction (scores are small), and a ones-column in V
to get softmax denominators from the AV matmul.  The multi-scale window fusion
is folded into pre-scaled projection-weight variants and combined biases
(computed on host).
"""
import sys
sys.path.insert(0, '/opt/trn_rl_repo')
import numpy as np
from contextlib import ExitStack

import ml_dtypes
import concourse.bass as bass
import concourse.bacc as bacc
import concourse.tile as tile
from concourse import mybir
from concourse.bass_utils import run_bass_kernel_spmd

F32 = mybir.dt.float32
BF16 = mybir.dt.bfloat16
F8 = mybir.dt.float8e4
DR = mybir.MatmulPerfMode.DoubleRow
AF = mybir.ActivationFunctionType
OP = mybir.AluOpType
P = 128
WS = 2048.0          # fp8 weight scale (power of 2); weights ~0.02*randn
WSI = 1.0 / WS
AS = 16.0            # fp8 activation scale (folded into LN rstd)
ASI = 1.0 / AS

B, T, K, C = 4, 16, 196, 768
NH, HD, MLP = 12, 64, 3072
SC = 98              # seqs per core (temporal)
NT = SC * T          # 1568 temporal tokens per core
NS = 8 * (K + 1)     # 1576 spatial tokens per core (8 t x 197)
NM = NS + 1          # 1577 mlp rows (last = true CLS row)
C8_0 = 8 * SC        # 784: first (t,s) col of the x8 range
C4_0 = 12 * SC       # 1176: first col of the x4 range
N_CORES = 8
EPS = 1e-5
SCALE = HD ** -0.5

_cache = {}

# temporal groups: 12 groups of 8 seqs + 1 tail group of 2 seqs; token col
# (group-major) = gbase + t*nseq + s.  scale-8 local col = g8 + (t-8)*nseq + s.
GROUPS = [(g, g * 128, 8) for g in range(12)] + [(12, 1536, 2)]
G8BASE = [g * 64 for g in range(12)] + [768]
G4BASE = [g * 32 for g in range(12)] + [384]


def row_tiles(n):
    return [(i, i * P, min(P, n - i * P)) for i in range((n + P - 1) // P)]


# ---------------------------------------------------------------------------
# emission helpers
# ---------------------------------------------------------------------------

def emit_ln(nc, pools, n_rows, load_tile_fn, out_fm, ident, out_lo=None):
    """LayerNorm (no affine, eps=1e-5) over C=768, scaled by AS; out
    feature-major fp8 (hi) + optional fp8 residual (lo, same scale).

    load_tile_fn(ti, r0, nr) -> [128, 768] f32 sbuf view.
    out_fm: [128, 6, ncols] (c = kc*128 + p, col = token index).
    """
    stat, tmp, psT = pools['stat'], pools['tmp'], pools['psT']
    eps2 = pools['eps2']
    for ti, r0, nr in row_tiles(n_rows):
        xt = load_tile_fn(ti, r0, nr)
        st = stat.tile([P, 3, 6], F32, tag="ln_st")
        xr = xt[:nr].rearrange("p (a b) -> p a b", a=3)
        for sg in range(3):
            nc.vector.bn_stats(out=st[:nr, sg, :], in_=xr[:, sg, :])
        mv = stat.tile([P, 2], F32, tag="ln_mv")
        nc.vector.bn_aggr(out=mv[:nr], in_=st[:nr])
        rstd = stat.tile([P, 1], F32, tag="ln_rstd")
        nc.scalar.activation(out=rstd[:nr], in_=mv[:nr, 1:2], func=AF.Sqrt,
                             bias=eps2[:nr], scale=1.0 / (AS * AS))
        nc.vector.reciprocal(out=rstd[:nr], in_=rstd[:nr])
        xn = tmp.tile([P, C], BF16, tag="ln_out")
        nc.vector.tensor_scalar(out=xn[:nr], in0=xt[:nr],
                                scalar1=mv[:nr, 0:1], scalar2=rstd[:nr],
                                op0=OP.subtract, op1=OP.mult)
        for kc in range(6):
            pt = psT.tile([P, P], BF16, tag="ln_tp")
            nc.tensor.transpose(pt[:, :nr], xn[:nr, kc * P:(kc + 1) * P],
                                ident[:nr, :nr])
            nc.any.tensor_copy(out=out_fm[:, kc, r0:r0 + nr], in_=pt[:, :nr])
            if out_lo is not None:
                nc.any.tensor_sub(out=out_lo[:, kc, r0:r0 + nr],
                                  in0=pt[:, :nr],
                                  in1=out_fm[:, kc, r0:r0 + nr])


def emit_gemm_fm(nc, pools, w_sb, act_fm, ncols, mtiles, evict_fn,
                 act_col_off=0, psum_tag="gfm"):
    """feature-major GEMM: psum[m, col] = sum_kc w[:,kc,m].T @ act[:,kc,col]."""
    ps = pools['ps']
    for mi in range(mtiles):
        for c0 in range(0, ncols, 512):
            ncl = min(512, ncols - c0)
            pp = ps.tile([P, 512], F32, tag=psum_tag)
            for kc in range(6):
                nc.tensor.matmul(
                    pp[:, :ncl],
                    w_sb[:, kc, mi * P:(mi + 1) * P],
                    act_fm[:, kc, act_col_off + c0:act_col_off + c0 + ncl],
                    start=(kc == 0), stop=(kc == 5))
            evict_fn(mi, c0, ncl, pp[:, :ncl])


def build_program(debug=False, n_reps=1, single=False):
    ncores = 1 if single else N_CORES
    nc = bacc.Bacc("TRN2", target_bir_lowering=False, debug=False,
                   enable_asserts=True, num_devices=ncores)
    env = {'debug': debug, 'nc': nc, 'single': single}

    def inp(name, shape, dt=BF16):
        h = nc.dram_tensor(name, shape, dt, kind="ExternalInput")
        env[name + '_d'] = h
        return h

    inp("xt", [NT, C], F32)
    inp("xtb", [NT, C], F32)
    inp("cls", [1, C], F32)
    inp("mth", [P, 2], F32)
    inp("mb", [P, B], F32)
    for s in ("16", "8", "4"):
        inp(f"wqk{s}", [C, 2 * C], F8); inp(f"wv{s}", [C, C], F8)
    for nm in ("wp16a", "wp16b", "wp8b", "wp8c", "wp4c"):
        inp(nm, [C, C], F8)
    inp("bias_abc", [3, C], F32)
    inp("wtfc", [C, C], F8)
    inp("wqks", [C, 2 * C], F8); inp("wvs", [C, C], F8)
    inp("wqks_lo", [C, 2 * C], F8); inp("wvs_lo", [C, C], F8)
    inp("wprojs", [C, C], BF16); inp("bprojs_b", [P, C], F32)
    inp("wfc1", [C, MLP], F8); inp("wfc1_lo", [C, MLP], F8)
    inp("bfc1", [1, MLP], F32)
    inp("wfc2", [MLP, C], F8); inp("wfc2_lo", [MLP, C], F8)
    inp("bfc2b", [P, C], F32)

    identf_np = np.eye(P, dtype=np.float32)
    identb_np = np.eye(P, dtype=ml_dtypes.bfloat16)
    mask8_np = np.where((np.arange(P)[:, None] % 8) == (np.arange(P)[None, :] % 8),
                        1.0, 0.0).astype(ml_dtypes.bfloat16)
    mask2_np = np.where((np.arange(32)[:, None] % 2) == (np.arange(32)[None, :] % 2),
                        1.0, 0.0).astype(ml_dtypes.bfloat16)
    env['identf_c'] = nc.inline_tensor(identf_np, name="identfc")
    env['identb_c'] = nc.inline_tensor(identb_np, name="identbc")
    env['mask8_c'] = nc.inline_tensor(mask8_np, name="mask8c")
    env['mask2_c'] = nc.inline_tensor(mask2_np, name="mask2c")

    env['out_d'] = nc.dram_tensor("out", [NM, C], F32, kind="ExternalOutput")
    dbg = {}
    if debug:
        for nm, shp in [("d_xtfull", [NT, C]), ("d_xspre", [NS, C]),
                        ("d_ressp", [NS, C]), ("d_clsagg", [1, C]),
                        ("d_xcat", [NM, C])]:
            dbg[nm] = nc.dram_tensor(nm, shp, F32, kind="ExternalOutput")
    env['dbg'] = dbg

    with tile.TileContext(nc) as tc, ExitStack() as ctx:
        env['tc'] = tc
        consts = ctx.enter_context(tc.tile_pool(name="consts", bufs=1))
        tmp = ctx.enter_context(tc.tile_pool(name="tmp", bufs=4))
        stat = ctx.enter_context(tc.tile_pool(name="stat", bufs=4))
        small = ctx.enter_context(tc.tile_pool(name="small", bufs=1))
        psM = ctx.enter_context(tc.tile_pool(name="psM", bufs=2, space="PSUM"))
        psS = ctx.enter_context(tc.tile_pool(name="psS", bufs=2, space="PSUM"))
        psO = ctx.enter_context(tc.tile_pool(name="psO", bufs=2, space="PSUM"))
        psT = ctx.enter_context(tc.tile_pool(name="psT", bufs=2, space="PSUM"))
        dram = ctx.enter_context(tc.tile_pool(name="dram", bufs=1, space="DRAM"))
        env['pools'] = {'ps': psM, 'psT': psT, 'psA': psS, 'psO': psO,
                        'stat': stat, 'tmp': tmp, 'small': small}

        for nm, src, shp, dt in [
                ('identf', 'identf_c', [P, P], F32),
                ('identb', 'identb_c', [P, P], BF16),
                ('mask8', 'mask8_c', [P, P], BF16),
                ('mask2', 'mask2_c', [32, 32], BF16)]:
            t = consts.tile(shp, dt, tag=nm, name=nm)
            nc.sync.dma_start(t, env[src].ap())
            env[nm] = t
        for nm, src, shp in [('mth', 'mth_d', [P, 2]), ('mb', 'mb_d', [P, B]),
                             ('cls_sb', 'cls_d', [1, C]),
                             ('bprojs_b', 'bprojs_b_d', [P, C]),
                             ('bfc2b', 'bfc2b_d', [P, C])]:
            t = consts.tile(shp, F32, tag=nm, name=nm)
            nc.sync.dma_start(t, env[src].ap())
            env[nm] = t
        epst = consts.tile([P, 1], F32, tag="eps", name="epst")
        nc.vector.memset(epst, EPS)
        env['eps'] = epst
        env['pools']['eps'] = epst
        eps2t = consts.tile([P, 1], F32, tag="eps2", name="eps2t")
        nc.vector.memset(eps2t, EPS / (AS * AS))
        env['pools']['eps2'] = eps2t
        t = consts.tile([P, 3, 6], F32, tag="bias_abc")
        nc.sync.dma_start(t, env['bias_abc_d'].ap().rearrange(
            "a (kc p) -> p a kc", p=P))
        env['bias_abc'] = t
        t = consts.tile([P, 24], F32, tag="bfc1")
        nc.sync.dma_start(t, env['bfc1_d'].ap().rearrange(
            "o (kc p) -> p (o kc)", p=P))
        env['bfc1'] = t

        def load_w(pool, name, cols, tag, bufs=1, dt=F8):
            w = pool.tile([P, 6, cols], dt, tag=tag, name="w_" + name,
                          bufs=bufs)
            nc.sync.dma_start(w, env[name + '_d'].ap().rearrange(
                "(kc p) n -> p kc n", p=P))
            return w
        env['load_w'] = load_w
        env['xcat_d'] = dram.tile([NM, C], F32, name='xcat_d')

        env['bounce'] = dram.tile([NT, C], F32, name='bounce')
        env['gath'] = dram.tile([2, NT, C], F32, name='gath')
        env['cls_bounce'] = dram.tile([8, C], F32, name='cls_bounce')
        env['cls_gath'] = nc.dram_tensor("clsg", [8 * N_CORES, C], F32,
                                         addr_space="Shared")
        env['xspre_d'] = dram.tile([NS, C], F32, name='xspre')

        for _rep in range(n_reps):
            emit_body(nc, env)
        _cache['marks'] = list(env.get('marks', []))

    nc.compile()
    return nc, dbg


def emit_body(nc, env):
    g = env.__getitem__
    tc = g('tc')
    pools = g('pools')
    tmp, stat, small = pools['tmp'], pools['stat'], pools['small']
    psM, psS, psO, psT = pools['ps'], pools['psA'], pools['psO'], pools['psT']
    identf, identb, mask8, mask2 = g('identf'), g('identb'), g('mask8'), g('mask2')
    mth, mb, cls_sb = g('mth'), g('mb'), g('cls_sb')
    bias_abc, bprojs_b, bfc1, bfc2b = (g('bias_abc'), g('bprojs_b'), g('bfc1'),
                                       g('bfc2b'))
    load_w = g('load_w')
    bounce, gath, cls_bounce, cls_gath, xspre_d, xcat_d = (
        g('bounce'), g('gath'), g('cls_bounce'), g('cls_gath'), g('xspre_d'),
        g('xcat_d'))
    out_d, dbg, debug = g('out_d'), g('dbg'), g('debug')

    marks = env.setdefault('marks', [])

    def mark(lbl):
        marks.append((lbl, nc.next_id()))
    mark('t_ln')

    # ================= temporal stage =================
    with tc.tile_pool(name="TPo", bufs=1) as TPo:
        o = {}
        for nm, cols in (("16", NT), ("8", 784), ("4", 392)):
            o[nm] = TPo.tile([P, 6, cols], F8, tag=f"o{nm}", name=f"o{nm}")

        with tc.tile_pool(name="TPln", bufs=1) as TPln:
            xtn = TPln.tile([P, 6, NT], F8, tag="xtn")

            def load_xt(ti, r0, nr):
                t = tmp.tile([P, C], F32, tag="t768")
                nc.sync.dma_start(t[:nr], g('xt_d').ap()[r0:r0 + nr])
                return t
            emit_ln(nc, pools, NT, load_xt, xtn, identb)

            # block-rotated qk buffers: all scales in one scope; attention of
            # block b overlaps the qk GEMM of block b+1.
            sc_tab = (("16", 0, 16, lambda gi: GROUPS[gi][1]),
                      ("8", 8, 8, lambda gi: G8BASE[gi]),
                      ("4", 12, 4, lambda gi: G4BASE[gi]))
            gblocks = [list(range(4 * i, 4 * i + 4)) for i in range(3)] + [[12]]
            with tc.tile_pool(name="TPqk", bufs=1) as TPqk:
                for nm, tlo, nt_sc, lbase in sc_tab:
                    mark('t_scale' + nm)
                    w = load_w(TPqk, f'wqk{nm}', 2 * C, tag="wqk_t", bufs=2)
                    wv = load_w(TPqk, f'wv{nm}', C, tag="wv_t", bufs=2)
                    for gs in gblocks:
                        bw = sum(nt_sc * GROUPS[gi][2] for gi in gs)
                        c0 = GROUPS[gs[0]][1] + tlo * GROUPS[gs[0]][2]
                        # xtn source cols for this scale+block are contiguous
                        # only per group; emit per-group matmuls into one psum
                        qkb = TPqk.tile([P, 12, 512], BF16, tag="qkb",
                                        bufs=3, name="qkb")
                        for mi in range(12):
                            pp = psM.tile([P, 512], F32, tag="mm")
                            off = 0
                            offs = []
                            for gi in gs:
                                _, gb, nseq = GROUPS[gi]
                                wcols = nt_sc * nseq
                                for kj in range(3):
                                    nc.tensor.matmul(
                                        pp[:, off:off + wcols],
                                        w[:, 2 * kj:2 * kj + 2,
                                          mi * P:(mi + 1) * P],
                                        xtn[:, 2 * kj:2 * kj + 2,
                                            gb + tlo * nseq:
                                            gb + tlo * nseq + wcols],
                                        start=(kj == 0), stop=(kj == 2),
                                        perf_mode=DR)
                                offs.append((off, wcols, gi))
                                off += wcols
                            nc.vector.tensor_copy(out=qkb[:, mi, :bw],
                                                  in_=pp[:, :bw])
                        boff = {gi: ofs for (ofs, wcols, gi) in offs}
                        for gi in gs:
                            _, gb, nseq = GROUPS[gi]
                            kp = nt_sc * nseq
                            src0 = gb + tlo * nseq
                            va = tmp.tile([P, 12, HD + 1], BF16, tag="vaug")
                            for nb in range(2):
                                pv = psM.tile([P, 512], F32, tag="mm")
                                for kj in range(3):
                                    nc.tensor.matmul(
                                        pv[:kp, :384],
                                        xtn[:, 2 * kj:2 * kj + 2,
                                            src0:src0 + kp],
                                        wv[:, 2 * kj:2 * kj + 2,
                                           nb * 384:(nb + 1) * 384],
                                        start=(kj == 0), stop=(kj == 2),
                                        perf_mode=DR)
                                nc.vector.tensor_copy(
                                    out=va[:kp, 6 * nb:6 * nb + 6, 0:HD],
                                    in_=pv[:kp, :384].rearrange(
                                        "p (h d) -> p h d", d=HD))
                            nc.vector.memset(va[:kp, :, HD:HD + 1], WS)
                            lq0 = boff[gi]
                            lc0 = lbase(gi)
                            msk = (mask8 if nseq == 8 else mask2)[:kp, :kp]
                            for h in range(NH):
                                mt_q, ro = h // 2, (h % 2) * HD
                                pS = psS.tile([P, 256], F32, tag="sps")
                                nc.tensor.matmul(
                                    pS[:kp, :kp],
                                    qkb[ro:ro + HD, 6 + mt_q, lq0:lq0 + kp],
                                    qkb[ro:ro + HD, mt_q, lq0:lq0 + kp],
                                    start=True, stop=True)
                                pt = tmp.tile([P, 256], BF16, tag="ptile")
                                nc.scalar.activation(out=pt[:kp, :kp],
                                                     in_=pS[:kp, :kp],
                                                     func=AF.Exp, scale=SCALE * (WSI * ASI) ** 2)
                                nc.vector.tensor_mul(out=pt[:kp, :kp],
                                                     in0=pt[:kp, :kp], in1=msk)
                                pO = psO.tile([HD + 1, 256], F32, tag="ops")
                                nc.tensor.matmul(pO[:, :kp], va[:kp, h, :],
                                                 pt[:kp, :kp], start=True,
                                                 stop=True)
                                rec = small.tile([1, 256], F32, tag="rec",
                                                 bufs=3)
                                nc.vector.reciprocal(out=rec[:, :kp],
                                                     in_=pO[HD:HD + 1, :kp])
                                rb = small.tile([HD, 256], F32, tag="rb",
                                                bufs=3)
                                nc.gpsimd.partition_broadcast(rb[:, :kp],
                                                              rec[:, :kp])
                                nc.vector.tensor_mul(
                                    out=o[nm][ro:ro + HD, mt_q, lc0:lc0 + kp],
                                    in0=pO[0:HD, :kp], in1=rb[:, :kp])

        with tc.tile_pool(name="TPpr", bufs=1) as TPpr:
            mark('t_proj')
            wp = {}
            for nm in ("wp16a", "wp16b", "wp8b", "wp8c", "wp4c"):
                wp[nm] = load_w(TPpr, nm, C, tag=f"wp_{nm}")
            res = TPpr.tile([P, 6, NT], F8, tag="res")
            # per (mtile, 4-group block): accumulate the three scales in psum,
            # then evict with the 3 range biases per group.
            for mi in range(6):
                for g0 in range(0, 13, 4):
                    gs = list(range(g0, min(g0 + 4, 13)))
                    blk0 = GROUPS[g0][1]
                    blkw = sum(16 * GROUPS[gi][2] for gi in gs)
                    pp = psM.tile([P, 512], F32, tag="mm")
                    mms = []
                    for gi in gs:
                        _, gb, nseq = GROUPS[gi]
                        ofs = gb - blk0
                        # x16 full group (bias range a for t<8, b for t>=8)
                        mms.append((wp['wp16a'], o['16'], gb, ofs, 8 * nseq))
                        mms.append((wp['wp16b'], o['16'], gb + 8 * nseq,
                                    ofs + 8 * nseq, 8 * nseq))
                        mms.append((wp['wp8b'], o['8'], G8BASE[gi],
                                    ofs + 8 * nseq, 4 * nseq))
                        mms.append((wp['wp8c'], o['8'], G8BASE[gi] + 4 * nseq,
                                    ofs + 12 * nseq, 4 * nseq))
                        mms.append((wp['wp4c'], o['4'], G4BASE[gi],
                                    ofs + 12 * nseq, 4 * nseq))
                    n_mm = 3 * len(mms)
                    i_mm = 0
                    for (w, osrc, sc0, dof, wcols) in mms:
                        first = w is wp['wp16a'] or w is wp['wp16b']
                        for kj in range(3):
                            nc.tensor.matmul(
                                pp[:, dof:dof + wcols],
                                w[:, 2 * kj:2 * kj + 2, mi * P:(mi + 1) * P],
                                osrc[:, 2 * kj:2 * kj + 2, sc0:sc0 + wcols],
                                start=(first and kj == 0),
                                stop=(i_mm == n_mm - 1),
                                skip_group_check=True, perf_mode=DR)
                            i_mm += 1
                    full = [gi for gi in gs if GROUPS[gi][2] == 8]
                    for rng, t0, t1 in ((0, 0, 8), (1, 8, 12), (2, 12, 16)):
                        if full:
                            nf = len(full)
                            rv = res[:, mi, 0:1536].rearrange(
                                "p (gg x) -> p gg x", x=P)
                            pv = pp[:, 0:512].rearrange(
                                "p (gg x) -> p gg x", x=P)
                            nc.scalar.activation(
                                out=rv[:, full[0]:full[0] + nf,
                                       t0 * 8:t1 * 8],
                                in_=pv[:, (GROUPS[full[0]][1] - blk0) // P:
                                       (GROUPS[full[0]][1] - blk0) // P + nf,
                                       t0 * 8:t1 * 8],
                                func=AF.Identity, scale=WSI,
                                bias=bias_abc[:, rng, mi:mi + 1])
                        for gi in gs:
                            _, gb, nseq = GROUPS[gi]
                            if nseq == 8:
                                continue
                            ofs = gb - blk0
                            nc.scalar.activation(
                                out=res[:, mi, gb + t0 * nseq:gb + t1 * nseq],
                                in_=pp[:, ofs + t0 * nseq:ofs + t1 * nseq],
                                func=AF.Identity, scale=WSI,
                                bias=bias_abc[:, rng, mi:mi + 1])

            mark('t_tfc')
            wtfc = load_w(TPpr, 'wtfc', C, tag="wtfc")
            for ti, r0, nr in row_tiles(NT):
                xb = tmp.tile([P, C], F32, tag="t768")
                nc.sync.dma_start(xb[:nr], g('xtb_d').ap()[r0:r0 + nr])
                xf = tmp.tile([P, C], F32, tag="t768")
                for nb in range(2):
                    pt = psM.tile([P, 512], F32, tag="mm")
                    for kj in range(3):
                        nc.tensor.matmul(pt[:nr, :384],
                                         res[:, 2 * kj:2 * kj + 2, r0:r0 + nr],
                                         wtfc[:, 2 * kj:2 * kj + 2,
                                              nb * 384:(nb + 1) * 384],
                                         start=(kj == 0), stop=(kj == 2),
                                         perf_mode=DR)
                    nc.vector.scalar_tensor_tensor(
                        out=xf[:nr, nb * 384:(nb + 1) * 384], in0=pt[:nr, :384],
                        scalar=WSI * ASI, in1=xb[:nr, nb * 384:(nb + 1) * 384],
                        op0=OP.mult, op1=OP.add)
                nc.sync.dma_start(bounce[r0:r0 + nr], xf[:nr])
                if debug:
                    nc.sync.dma_start(dbg['d_xtfull'].ap()[r0:r0 + nr],
                                      xf[:nr])

    mark('exchange')
    if g('single'):
        for rk in range(2):
            nc.sync.dma_start(gath[rk], bounce[:])
    else:
        nc.gpsimd.collective_compute(
            "AllGather", OP.bypass,
            replica_groups=[[0, 1], [2, 3], [4, 5], [6, 7]],
            ins=[bounce.opt()], outs=[gath.opt()])

    # ============ xs_pre (DRAM): rows (tl, 0)=cls, (tl, 1+k)=xt_full ============
    mark('xspre')
    _XS_cm = tc.tile_pool(name="XS", bufs=4)
    XS = _XS_cm.__enter__()
    for tl in range(8):
        nc.sync.dma_start(xspre_d[tl * 197:tl * 197 + 1], cls_sb)
        for rank in range(2):
            # candidate rows for t = tl (lo) and t = 8+tl (hi); bounce rows
            # are group-major: row = g*128 + t*nseq + s  (k = 8g+s ascending)
            lo = XS.tile([SC, C], F32, tag="sel_lo", bufs=4)
            hi = XS.tile([SC, C], F32, tag="sel_hi", bufs=4)
            gmain = gath[rank][0:1536].rearrange("(gg x) c -> gg x c", x=P)
            for dst, t in ((lo, tl), (hi, 8 + tl)):
                nc.sync.dma_start(dst[:96], gmain[:, t * 8:t * 8 + 8, :])
                nc.sync.dma_start(dst[96:98],
                                  gath[rank][1536 + t * 2:1536 + t * 2 + 2])
            selt = XS.tile([SC, C], F32, tag="sel_out", bufs=4)
            nc.vector.tensor_scalar_mul(out=selt, in0=hi,
                                        scalar1=mth[:SC, 1:2])
            nc.vector.scalar_tensor_tensor(out=selt, in0=lo,
                                           scalar=mth[:SC, 0:1], in1=selt,
                                           op0=OP.mult, op1=OP.add)
            nc.sync.dma_start(
                xspre_d[tl * 197 + 1 + rank * SC:tl * 197 + 1 + rank * SC + SC],
                selt)
    _XS_cm.__exit__(None, None, None)
    if debug:
        for ti, r0, nr in row_tiles(NS):
            t = tmp.tile([P, C], F32, tag="t768")
            nc.sync.dma_start(t[:nr], xspre_d[r0:r0 + nr])
            nc.sync.dma_start(dbg['d_xspre'].ap()[r0:r0 + nr], t[:nr])

    # ================= spatial stage =================
    mark('s_ln')
    with tc.tile_pool(name="SP1", bufs=1) as SP1:
        ressp = SP1.tile([P, 13, C], F32, tag="ressp")
        with tc.tile_pool(name="SPo", bufs=1) as SPo:
            os_ = SPo.tile([P, 6, NS], BF16, tag="osp", name="os_")
            with tc.tile_pool(name="SPln", bufs=1) as SPln:
                # free-dim padded to a multiple of 16: dual-fp8 ldweights
                # requires 16B-aligned k-subtile strides (NS=1576 is not)
                xsn = SPln.tile([P, 6, NS + 8], F8, tag="xsn")
                xsn_lo = SPln.tile([P, 6, NS + 8], F8, tag="xsn_lo")

                def load_xs(ti, r0, nr):
                    t = tmp.tile([P, C], F32, tag="t768")
                    nc.sync.dma_start(t[:nr], xspre_d[r0:r0 + nr])
                    return t
                emit_ln(nc, pools, NS, load_xs, xsn, identb, out_lo=xsn_lo)

                mark('s_qkv')
                with tc.tile_pool(name="SPqk", bufs=1) as SPqk:
                    w = load_w(SPqk, 'wqks', 2 * C, tag="wqk_t", bufs=1)
                    wlo = load_w(SPqk, 'wqks_lo', 2 * C, tag="wqk_lo", bufs=1)
                    wvs = load_w(SPqk, 'wvs', C, tag="wv_t", bufs=1)
                    wvs_lo = load_w(SPqk, 'wvs_lo', C, tag="wv_lo", bufs=1)
                    for tl in range(8):
                        q0 = tl * 197
                        qkb = SPqk.tile([P, 12, 197], BF16, tag="qksb",
                                        bufs=3, name="qkb_sp")
                        for mi in range(12):
                            pp = psM.tile([P, 512], F32, tag="mm")
                            passes = [(w, xsn), (wlo, xsn), (w, xsn_lo)]
                            for pi, (wx, ax) in enumerate(passes):
                                for kj in range(3):
                                    nc.tensor.matmul(
                                        pp[:, :197],
                                        wx[:, 2 * kj:2 * kj + 2,
                                           mi * P:(mi + 1) * P],
                                        ax[:, 2 * kj:2 * kj + 2, q0:q0 + 197],
                                        start=(pi == 0 and kj == 0),
                                        stop=(pi == 2 and kj == 2),
                                        perf_mode=DR)
                            nc.vector.tensor_copy(out=qkb[:, mi, :],
                                                  in_=pp[:, :197])
                        va = tmp.tile([P, 2, 12, HD + 1], BF16, tag="vaug")
                        for kt in range(2):
                            kp = 128 if kt == 0 else 69
                            for nb in range(2):
                                pv = psM.tile([P, 512], F32, tag="mm")
                                vpasses = [(xsn, wvs), (xsn, wvs_lo),
                                           (xsn_lo, wvs)]
                                for pi, (ax, wx) in enumerate(vpasses):
                                    for kj in range(3):
                                        nc.tensor.matmul(
                                            pv[:kp, :384],
                                            ax[:, 2 * kj:2 * kj + 2,
                                               q0 + kt * P:q0 + kt * P + kp],
                                            wx[:, 2 * kj:2 * kj + 2,
                                               nb * 384:(nb + 1) * 384],
                                            start=(pi == 0 and kj == 0),
                                            stop=(pi == 2 and kj == 2),
                                            perf_mode=DR)
                                nc.vector.tensor_copy(
                                    out=va[:kp, kt, 6 * nb:6 * nb + 6, 0:HD],
                                    in_=pv[:kp, :384].rearrange(
                                        "p (h d) -> p h d", d=HD))
                            nc.vector.memset(va[:kp, kt, :, HD:HD + 1], WS * AS)
                        for h in range(NH):
                            mt_q, ro = h // 2, (h % 2) * HD
                            pO = psO.tile([HD + 1, 256], F32, tag="ops")
                            for kt in range(2):
                                kp = 128 if kt == 0 else 69
                                pS = psS.tile([P, 256], F32, tag="sps")
                                nc.tensor.matmul(
                                    pS[:kp, :197],
                                    qkb[ro:ro + HD, 6 + mt_q,
                                        kt * P:kt * P + kp],
                                    qkb[ro:ro + HD, mt_q, :],
                                    start=True, stop=True)
                                pt = tmp.tile([P, 256], BF16, tag="ptile")
                                nc.scalar.activation(out=pt[:kp, :197],
                                                     in_=pS[:kp, :197],
                                                     func=AF.Exp, scale=SCALE * (WSI * ASI) ** 2)
                                nc.tensor.matmul(pO[:, :197],
                                                 va[:kp, kt, h, :],
                                                 pt[:kp, :197],
                                                 start=(kt == 0),
                                                 stop=(kt == 1))
                            rec = small.tile([1, 256], F32, tag="rec", bufs=3)
                            nc.vector.reciprocal(out=rec[:, :197],
                                                 in_=pO[HD:HD + 1, :197])
                            rb = small.tile([HD, 256], F32, tag="rb", bufs=3)
                            nc.gpsimd.partition_broadcast(rb[:, :197],
                                                          rec[:, :197])
                            nc.vector.tensor_mul(
                                out=os_[ro:ro + HD, mt_q, q0:q0 + 197],
                                in0=pO[0:HD, :197], in1=rb[:, :197])

            mark('s_proj')
            with tc.tile_pool(name="SPp", bufs=1) as SPp:
                wprojs = load_w(SPp, 'wprojs', C, tag="wproj", dt=BF16)
                for ti, r0, nr in row_tiles(NS):
                    for nb in range(2):
                        pp = psM.tile([P, 512], F32, tag="mm")
                        for kc in range(6):
                            nc.tensor.matmul(
                                pp[:nr, :384], os_[:, kc, r0:r0 + nr],
                                wprojs[:, kc, nb * 384:(nb + 1) * 384],
                                start=(kc == 0), stop=(kc == 5))
                        nc.vector.tensor_add(
                            out=ressp[:nr, ti, nb * 384:(nb + 1) * 384],
                            in0=pp[:nr, :384],
                            in1=bprojs_b[:nr, nb * 384:(nb + 1) * 384])
                if debug:
                    for ti, r0, nr in row_tiles(NS):
                        nc.sync.dma_start(dbg['d_ressp'].ap()[r0:r0 + nr],
                                          ressp[:nr, ti, :])

        # ================= CLS aggregation =================
        mark('cls')
        for tl in range(8):
            r = tl * 197
            nc.sync.dma_start(cls_bounce[tl:tl + 1],
                              ressp[r % P:r % P + 1, r // P, :])
        if g('single'):
            for rk in range(8):
                nc.sync.dma_start(cls_gath.ap()[8 * rk:8 * rk + 8],
                                  cls_bounce[:])
        else:
            nc.gpsimd.collective_compute(
                "AllGather", OP.bypass,
                replica_groups=[list(range(N_CORES))],
                ins=[cls_bounce.opt()], outs=[cls_gath.ap()])
        cls_own = small.tile([16, C], F32, tag="clsown")
        for j in range(B):
            cj = tmp.tile([16, C], F32, tag="t768")
            nc.sync.dma_start(cj, cls_gath.ap()[16 * j:16 * j + 16])
            if j == 0:
                nc.vector.tensor_scalar_mul(out=cls_own, in0=cj,
                                            scalar1=mb[:16, 0:1])
            else:
                nc.vector.scalar_tensor_tensor(out=cls_own, in0=cj,
                                               scalar=mb[:16, j:j + 1],
                                               in1=cls_own, op0=OP.mult,
                                               op1=OP.add)
        st = stat.tile([16, 3, 6], F32, tag="ln_st")
        cr = cls_own.rearrange("p (a b) -> p a b", a=3)
        for sg in range(3):
            nc.vector.bn_stats(out=st[:, sg, :], in_=cr[:, sg, :])
        mv = stat.tile([16, 2], F32, tag="ln_mv")
        nc.vector.bn_aggr(out=mv, in_=st)
        rstd = stat.tile([16, 1], F32, tag="ln_rstd")
        nc.scalar.activation(out=rstd, in_=mv[:, 1:2], func=AF.Sqrt,
                             bias=g('eps')[:16])
        nc.vector.reciprocal(out=rstd, in_=rstd)
        clsn = small.tile([16, C], F32, tag="clsn")
        nc.vector.tensor_scalar(out=clsn, in0=cls_own, scalar1=mv[:, 0:1],
                                scalar2=rstd, op0=OP.subtract, op1=OP.mult)
        clsfm = small.tile([P, 6, 16], F32, tag="clsfm")
        for kc in range(6):
            ptc = psT.tile([P, 16], F32, tag="ln_tp")
            nc.tensor.transpose(ptc, clsn[:, kc * P:(kc + 1) * P],
                                identf[:16, :16])
            nc.vector.tensor_copy(out=clsfm[:, kc, :], in_=ptc)
        psc = psS.tile([1, 256], F32, tag="sps")
        for kc in range(6):
            nc.tensor.matmul(psc[:, :16], clsfm[:, kc, 15:16], clsfm[:, kc, :],
                             start=(kc == 0), stop=(kc == 5))
        mx = small.tile([1, 1], F32, tag="clsmx")
        nc.vector.reduce_max(out=mx, in_=psc[:, :16], axis=mybir.AxisListType.X)
        sc_sb = small.tile([1, 16], F32, tag="clssb")
        nc.vector.tensor_scalar_sub(out=sc_sb, in0=psc[:, :16], scalar1=mx)
        esum = small.tile([1, 1], F32, tag="clses")
        att = small.tile([1, 16], F32, tag="clsatt")
        nc.scalar.activation(out=att, in_=sc_sb, func=AF.Exp, accum_out=esum)
        nc.vector.reciprocal(out=esum, in_=esum)
        nc.vector.tensor_scalar_mul(out=att, in0=att, scalar1=esum)
        attT_ps = psT.tile([16, 16], F32, tag="ln_tp")
        nc.tensor.transpose(attT_ps[:, :1], att, identf[:1, :1])
        attT = small.tile([16, 1], F32, tag="attTs")
        nc.vector.tensor_copy(out=attT, in_=attT_ps[:, :1])
        clsrow = small.tile([1, C], F32, tag="clsrow")
        for nb in range(2):
            pagg = psM.tile([P, 512], F32, tag="mm")
            nc.tensor.matmul(pagg[:1, :384], attT,
                             clsn[:, nb * 384:(nb + 1) * 384],
                             start=True, stop=True)
            nc.vector.tensor_add(out=clsrow[:, nb * 384:(nb + 1) * 384],
                                 in0=pagg[:1, :384],
                                 in1=cls_sb[:, nb * 384:(nb + 1) * 384])
        if debug:
            nc.sync.dma_start(dbg['d_clsagg'].ap(), clsrow)

        mark('xcat')
        # ========== x_cat -> DRAM ==========
        for ti, r0, nr in row_tiles(NS):
            t = tmp.tile([P, C], F32, tag="t768")
            nc.sync.dma_start(t[:nr], xspre_d[r0:r0 + nr])
            xc = tmp.tile([P, C], F32, tag="t768")
            nc.vector.tensor_add(out=xc[:nr], in0=t[:nr], in1=ressp[:nr, ti, :])
            nc.sync.dma_start(xcat_d[r0:r0 + nr], xc[:nr])
        nc.sync.dma_start(xcat_d[NM - 1:NM], clsrow)
    if debug:
        for ti, r0, nr in row_tiles(NM):
            t = tmp.tile([P, C], F32, tag="t768")
            nc.sync.dma_start(t[:nr], xcat_d[r0:r0 + nr])
            nc.sync.dma_start(dbg['d_xcat'].ap()[r0:r0 + nr], t[:nr])

    # ================= MLP =================
    mark('m_ln')
    with tc.tile_pool(name="ML", bufs=1) as ML:
        wfc1 = load_w(ML, 'wfc1', MLP, tag="wfc1")
        wfc1_lo = load_w(ML, 'wfc1_lo', MLP, tag="wfc1_lo")
        wfc2 = ML.tile([P, 24, C], F8, tag="wfc2", name="wfc2")
        nc.sync.dma_start(wfc2, g('wfc2_d').ap().rearrange(
            "(kc p) n -> p kc n", p=P))
        wfc2_lo = ML.tile([P, 24, C], F8, tag="wfc2_lo", name="wfc2_lo")
        nc.sync.dma_start(wfc2_lo, g('wfc2_lo_d').ap().rearrange(
            "(kc p) n -> p kc n", p=P))
        halves = [(0, 544), (544, 544), (1088, NM - 1088)]
        for hi0, hw in halves:
            hn = ML.tile([P, 6, 544], F8, tag="hn", bufs=2, name="hn")
            hn_lo = ML.tile([P, 6, 544], F8, tag="hn_lo", bufs=2, name="hn_lo")

            def load_xc(ti, r0, nr, hi0=hi0):
                t = tmp.tile([P, C], F32, tag="t768")
                nc.sync.dma_start(t[:nr], xcat_d[hi0 + r0:hi0 + r0 + nr])
                return t
            emit_ln(nc, pools, hw, load_xc, hn, identb, out_lo=hn_lo)

            gfm = ML.tile([P, 24, 544], F8, tag="gfm", bufs=2, name="gfm")
            for mi in range(24):
                for c0 in range(0, hw, 512):
                    ncl = min(512, hw - c0)
                    pg = psM.tile([P, 512], F32, tag="mm")
                    fpasses = [(wfc1, hn), (wfc1_lo, hn), (wfc1, hn_lo)]
                    for pi, (wx, ax) in enumerate(fpasses):
                        for kj in range(3):
                            nc.tensor.matmul(pg[:, :ncl],
                                             wx[:, 2 * kj:2 * kj + 2,
                                                mi * P:(mi + 1) * P],
                                             ax[:, 2 * kj:2 * kj + 2,
                                                c0:c0 + ncl],
                                             start=(pi == 0 and kj == 0),
                                             stop=(pi == 2 and kj == 2),
                                             perf_mode=DR)
                    nc.scalar.activation(out=gfm[:, mi, c0:c0 + ncl],
                                         in_=pg[:, :ncl], func=AF.Gelu,
                                         scale=WSI * ASI,
                                         bias=bfc1[:, mi:mi + 1])
            if hi0 == halves[-1][0]:
                mark('m_fc2')
            for ti, r0, nr in row_tiles(hw):
                xc = tmp.tile([P, C], F32, tag="t768")
                nc.sync.dma_start(xc[:nr], xcat_d[hi0 + r0:hi0 + r0 + nr])
                outt = tmp.tile([P, C], F32, tag="t768")
                for nb in range(2):
                    po = psM.tile([P, 512], F32, tag="mm")
                    for pi, wx in enumerate((wfc2, wfc2_lo)):
                        for kj in range(12):
                            nc.tensor.matmul(po[:nr, :384],
                                             gfm[:, 2 * kj:2 * kj + 2,
                                                 r0:r0 + nr],
                                             wx[:, 2 * kj:2 * kj + 2,
                                                nb * 384:(nb + 1) * 384],
                                             start=(pi == 0 and kj == 0),
                                             stop=(pi == 1 and kj == 11),
                                             perf_mode=DR)
                    sb = tmp.tile([P, 512], F32, tag="t768b")
                    nc.vector.scalar_tensor_tensor(
                        out=sb[:nr, :384], in0=po[:nr, :384], scalar=WSI,
                        in1=bfc2b[:nr, nb * 384:(nb + 1) * 384],
                        op0=OP.mult, op1=OP.add)
                    nc.vector.tensor_add(out=outt[:nr, nb * 384:(nb + 1) * 384],
                                         in0=sb[:nr, :384],
                                         in1=xc[:nr, nb * 384:(nb + 1) * 384])
                nc.sync.dma_start(out_d.ap()[hi0 + r0:hi0 + r0 + nr],
                                  outt[:nr])


# ---------------------------------------------------------------------------
# host side
# ---------------------------------------------------------------------------

def prep_inputs(inputs):
    x = np.asarray(inputs['x'], np.float32)
    f32 = lambda a: np.ascontiguousarray(np.asarray(a, np.float32))
    realbf = lambda a: np.ascontiguousarray(
        np.asarray(a, np.float32).astype(ml_dtypes.bfloat16))

    def tobf(a):  # single fp8 (scaled by WS)
        return np.ascontiguousarray(
            np.clip(np.asarray(a, np.float32) * WS, -240, 240)
            .astype(ml_dtypes.float8_e4m3))

    def tof8lo(a):  # fp8 residual of tobf(a), same WS scale
        a = np.asarray(a, np.float32) * WS
        hi = np.clip(a, -240, 240).astype(ml_dtypes.float8_e4m3)
        return np.ascontiguousarray(
            (a - hi.astype(np.float32)).astype(ml_dtypes.float8_e4m3))

    Wqkv = {s: np.asarray(inputs[f'Wqkv{s}'], np.float32) for s in (4, 8, 16)}
    Wp = {s: np.asarray(inputs[f'Wp{s}'], np.float32) for s in (4, 8, 16)}
    bp = {s: np.asarray(inputs[f'bp{s}'], np.float32) for s in (4, 8, 16)}
    Wqkv_s = np.asarray(inputs['Wqkv_s'], np.float32)
    shared = {
        'wqk16': tobf(Wqkv[16][:, :2 * C]), 'wv16': tobf(Wqkv[16][:, 2 * C:]),
        'wqk8': tobf(Wqkv[8][:, :2 * C]), 'wv8': tobf(Wqkv[8][:, 2 * C:]),
        'wqk4': tobf(Wqkv[4][:, :2 * C]), 'wv4': tobf(Wqkv[4][:, 2 * C:]),
        'wp16a': tobf(Wp[16]), 'wp16b': tobf(0.5 * Wp[16]),
        'wp8b': tobf(0.5 * Wp[8]), 'wp8c': tobf(0.25 * Wp[8]),
        'wp4c': tobf(0.25 * Wp[4]),
        'bias_abc': f32(AS * np.stack(
            [bp[16], 0.5 * bp[16] + 0.5 * bp[8],
             0.5 * bp[16] + 0.25 * bp[8] + 0.25 * bp[4]])),
        'wtfc': tobf(inputs['Wtfc']),
        'wqks': tobf(Wqkv_s[:, :2 * C]), 'wvs': tobf(Wqkv_s[:, 2 * C:]),
        'wqks_lo': tof8lo(Wqkv_s[:, :2 * C]),
        'wvs_lo': tof8lo(Wqkv_s[:, 2 * C:]),
        'wprojs': realbf(inputs['Wproj_s']),
        'bprojs_b': f32(np.tile(np.asarray(inputs['bproj_s'])[None, :], (P, 1))),
        'wfc1': tobf(inputs['Wfc1']), 'wfc1_lo': tof8lo(inputs['Wfc1']),
        'bfc1': f32(np.asarray(inputs['bfc1']).reshape(1, MLP)),
        'wfc2': tobf(inputs['Wfc2']), 'wfc2_lo': tof8lo(inputs['Wfc2']),
        'bfc2b': f32(np.tile(np.asarray(inputs['bfc2'])[None, :], (P, 1))),
    }
    btfc = np.asarray(inputs['btfc'], np.float32)
    perm = np.empty(NT, np.int64)
    for gi, gb, nseq in GROUPS:
        for t in range(T):
            for s in range(nseq):
                perm[gb + t * nseq + s] = (8 * gi + s) * T + t
    in_maps = []
    for core in range(N_CORES):
        b, half = core // 2, core % 2
        xs = x[b, 1 + half * NT: 1 + half * NT + NT]
        xt = xs[perm]
        mth = np.zeros((P, 2), np.float32); mth[:, half] = 1.0
        mbv = np.zeros((P, B), np.float32); mbv[:, b] = 1.0
        m = dict(shared)
        m.update({'xt': f32(xt), 'xtb': f32(xt + btfc),
                  'cls': f32(x[b, 0:1]), 'mth': mth, 'mb': mbv})
        in_maps.append(m)
    return in_maps


def assemble(results):
    out = np.zeros((B, 1 + K * T, C), np.float32)
    for core in range(N_CORES):
        b, half = core // 2, core % 2
        o = np.asarray(results[core]['out'])
        main = o[:NS].reshape(8, K + 1, C)[:, 1:, :]
        t = 8 * half + np.arange(8)
        idx = 1 + np.arange(K)[None, :] * T + t[:, None]
        out[b, idx.reshape(-1)] = main.reshape(-1, C)
        if half == 0:
            out[b, 0] = o[NM - 1]
    return out


def kernel(**inputs):
    if 'prog' not in _cache:
        _cache['prog'] = build_program(debug=False)
    nc, _ = _cache['prog']
    in_maps = prep_inputs(inputs)
    res = run_bass_kernel_spmd(nc, in_maps, list(range(N_CORES)))
    return assemble(res.results)

